# revision 15
# baseline (speedup 1.0000x reference)
"""2-layer GraphSAGE (mean aggr) on 8 Trainium2 NeuronCores — single launch.

Strategy: partition destination nodes across cores (graph parallel), both
layers fused into ONE program with an on-device AllGather exchanging the
hidden layer between them.

All tables live in "slot space": host permutes nodes into degree-balanced
slots (degree-sorted snake round-robin over blocks), core c owning slots
[c*spc, (c+1)*spc). Because both the L1 input table (xq) and the L2 input
table (h_full) are slot-ordered with the same chunking, one set of gather
indices / routing tensors serves both layers.

Per layer, per block of W=256 destination slots: gather tiles of 128
source rows M [128e, D] with dma_gather (int16 idx, 4 chunks of 25088
rows, round-robin over 4 SWDGE queues), build routing tile S [128e, W]
(one-hot by local destination x 1/deg) on DVE, accumulate agg[D, W] +=
M.T @ S on TensorE (fp32r, 1 cycle/row) in PSUM. Finalize twice: a
transposed [HID, W] tile (relu -> SBUF hT, the L2 self term) and a
row-major [W, HID] pair (relu -> DRAM h_local rows, the L2 gather table).
AllGather h_local [spc, D] -> h_full [8*spc, D], then L2 runs the same
block loop over h_full and writes out [64, spc] transposed; host
assembles/unpermutes.
"""

import contextlib
import sys

sys.path.insert(0, "/opt/trn_rl_repo")

import numpy as np

import concourse.mybir as mybir
import concourse.tile as tile
from concourse import bacc, bass_utils

N_NODES = 100000
N_EDGES = 1600000
IN_DIM = 128
HID_DIM = 128
OUT_DIM = 64
N_CORES = 8
N_CHUNKS = 4
W = 256               # destination slots per block (fp32r: >=256 moving rows)
GATHER_MAX = 1024     # HW limit: dma_gather wedges above this
N_QUEUES = 4          # SWDGE queues (ucode max 4)
MSG_BF16 = True       # message tables (xq, h) + routing tiles in bf16

_plan_cache: dict = {}
_prog_cache: dict = {}
_STAGE_MASK = "1c2"   # experiment hook: subset of stages to emit


def _make_plan(edge_index, n_nodes=N_NODES, n_cores=N_CORES,
               n_chunks=N_CHUNKS):
    src = np.asarray(edge_index[0], dtype=np.int64)
    dst = np.asarray(edge_index[1], dtype=np.int64)
    n_edges = src.shape[0]

    deg = np.bincount(dst, minlength=n_nodes).astype(np.int64)
    cnt_inv = (1.0 / np.maximum(deg, 1)).astype(np.float32)

    # Balanced blocks: degree-sorted snake round-robin over all blocks.
    n_blocks_total = -(-n_nodes // W)
    while n_blocks_total % n_cores:
        n_blocks_total += 1
    bpc = n_blocks_total // n_cores
    slots_per_core = bpc * W
    total_rows = n_blocks_total * W          # slot-space table height
    chunk_sz = total_rows // n_chunks
    assert chunk_sz - 1 <= np.iinfo(np.int16).max

    order = np.argsort(-deg, kind="stable")
    i = np.arange(n_nodes)
    r = i // n_blocks_total
    b = i % n_blocks_total
    b = np.where(r % 2 == 0, b, n_blocks_total - 1 - b)
    slot = b * W + r
    slot_of_node = np.empty(n_nodes, np.int64)
    slot_of_node[order] = slot

    sslot = slot_of_node[src]                # gather index, slot space
    dslot = slot_of_node[dst]
    core_e = dslot // slots_per_core
    blk_e = (dslot % slots_per_core) // W
    dloc_e = dslot % W
    chunk_e = sslot // chunk_sz

    cell = (core_e * bpc + blk_e) * n_chunks + chunk_e
    n_cells = n_cores * bpc * n_chunks
    counts = np.bincount(cell, minlength=n_cells).reshape(
        n_cores, bpc, n_chunks)
    T = -(-counts.max(axis=0) // 128)            # [bpc, n_chunks] tiles/cell

    cell_slots = (T * 128).astype(np.int64)
    seg_len = cell_slots.sum(axis=0)             # per chunk
    seg_start = np.concatenate([[0], np.cumsum(seg_len)[:-1]])
    cell_base = np.empty((bpc, n_chunks), np.int64)
    for c in range(n_chunks):
        cell_base[:, c] = seg_start[c] + np.concatenate(
            [[0], np.cumsum(cell_slots[:, c])[:-1]])
    total_slots = int(cell_slots.sum())

    gathers = []
    for c in range(n_chunks):
        lst = []
        off = 0
        while off < seg_len[c]:
            n = int(min(GATHER_MAX, seg_len[c] - off))
            lst.append((int(seg_start[c] + off), n))
            off += n
        gathers.append(lst)

    # slot position of every edge
    eorder = np.argsort(cell, kind="stable")
    sorted_cell = cell[eorder]
    group_start = np.zeros(n_edges, np.int64)
    new_grp = np.empty(n_edges, bool)
    new_grp[0] = True
    new_grp[1:] = sorted_cell[1:] != sorted_cell[:-1]
    grp_first = np.where(new_grp)[0]
    group_start[grp_first] = grp_first
    group_start = np.maximum.accumulate(group_start)
    rank = np.arange(n_edges) - group_start

    b_of = (sorted_cell // n_chunks) % bpc
    c_of = sorted_cell % n_chunks
    core_of = sorted_cell // (bpc * n_chunks)
    pos = cell_base[b_of, c_of] + rank

    idx_vals = np.zeros((n_cores, total_slots), np.int16)
    dloc_vals = np.full((n_cores, total_slots), -1.0, np.float32)
    cinv_vals = np.zeros((n_cores, total_slots), np.float32)

    es, ed = sslot[eorder], dst[eorder]
    idx_vals[core_of, pos] = (es - c_of * chunk_sz).astype(np.int16)
    dloc_vals[core_of, pos] = dloc_e[eorder].astype(np.float32)
    cinv_vals[core_of, pos] = cnt_inv[ed]

    idx16 = np.ascontiguousarray(
        np.tile(idx_vals.reshape(n_cores, -1, 16).transpose(0, 2, 1),
                (1, 8, 1)))
    dstloc = np.ascontiguousarray(
        dloc_vals.reshape(n_cores, -1, 128).transpose(0, 2, 1))
    cntinv = np.ascontiguousarray(
        cinv_vals.reshape(n_cores, -1, 128).transpose(0, 2, 1))

    return dict(
        slot_of_node=slot_of_node, bpc=bpc, slots_per_core=slots_per_core,
        total_rows=total_rows, chunk_sz=chunk_sz,
        T=T, gathers=gathers, total_slots=total_slots,
        cell_base=cell_base, seg_start=seg_start,
        idx16=idx16, dstloc=dstloc, cntinv=cntinv,
        n_chunks=n_chunks, n_nodes=n_nodes, n_cores=n_cores,
    )


def _build_program(plan, loop_k=1):
    """The fused two-layer SPMD program (shared by all cores).

    loop_k > 1 wraps the whole body (both layers + AllGather) in a
    hardware For loop repeating it loop_k times (timing only).
    """
    bpc = plan["bpc"]
    T = plan["T"]
    n_chunks = plan["n_chunks"]
    chunk_sz = plan["chunk_sz"]
    total_slots = plan["total_slots"]
    total_rows = plan["total_rows"]
    spc = plan["slots_per_core"]
    gathers = plan["gathers"]
    cell_base = plan["cell_base"]
    seg_start = plan["seg_start"]
    n_cores = plan["n_cores"]
    D = 128
    f32 = mybir.dt.float32
    mdt = mybir.dt.float32r
    tdt = mybir.dt.bfloat16 if MSG_BF16 else mdt   # gather-table dtype

    nc = bacc.Bacc("TRN2", target_bir_lowering=False, debug=False,
                   num_swdge_queues=N_QUEUES)
    with tile.TileContext(nc) as tc:
        with tc.tile_pool(name="dram", bufs=1, space="DRAM") as dram:
            xq = dram.tile([total_rows, D], tdt,
                           kind="ExternalInput", name="xq")
            idx16 = dram.tile([128, total_slots // 16], mybir.dt.int16,
                              kind="ExternalInput", name="idx16")
            dstloc = dram.tile([128, total_slots // 128], f32,
                               kind="ExternalInput", name="dstloc")
            cntinv = dram.tile([128, total_slots // 128], f32,
                               kind="ExternalInput", name="cntinv")
            xT = dram.tile([D, spc], f32, kind="ExternalInput", name="xT")
            w1l = dram.tile([D, HID_DIM], mdt,
                            kind="ExternalInput", name="w1l")
            w1r = dram.tile([D, HID_DIM], f32,
                            kind="ExternalInput", name="w1r")
            b1row = dram.tile([1, HID_DIM], mdt,
                              kind="ExternalInput", name="b1row")
            w2l = dram.tile([HID_DIM, OUT_DIM], mdt,
                            kind="ExternalInput", name="w2l")
            w2r = dram.tile([HID_DIM, OUT_DIM], f32,
                            kind="ExternalInput", name="w2r")
            b2row = dram.tile([1, OUT_DIM], mdt,
                              kind="ExternalInput", name="b2row")
            iota_in = dram.tile([128, W], tdt,
                                kind="ExternalInput", name="iota")
            onesr = dram.tile([1, W], mdt,
                              kind="ExternalInput", name="onesr")
            out = dram.tile([OUT_DIM, spc], f32,
                            kind="ExternalOutput", name="out")
            h_local = dram.tile([spc, D], tdt, name="h_local")
            h_full = dram.tile([total_rows, D], tdt, name="h_full")

        with tc.tile_pool(name="const", bufs=1) as cpool, \
             tc.tile_pool(name="gbuf", bufs=2) as gpool, \
             tc.tile_pool(name="spool", bufs=4) as spool, \
             tc.tile_pool(name="fpool", bufs=4) as fpool, \
             tc.tile_pool(name="psA", bufs=2, space="PSUM") as psA, \
             tc.tile_pool(name="psB", bufs=2, space="PSUM") as psB, \
             tc.tile_pool(name="psC", bufs=2, space="PSUM") as psC:

            idx_sb = cpool.tile([128, total_slots // 16], mybir.dt.int16)
            dst_sb = cpool.tile([128, total_slots // 128], f32)
            cnt_sb = cpool.tile([128, total_slots // 128], f32)
            xT_sb = cpool.tile([D, spc], f32)
            hT_sb = cpool.tile([D, spc], f32)
            w1l_sb = cpool.tile([D, HID_DIM], mdt)
            w1r_sb = cpool.tile([D, HID_DIM], f32)
            b1_sb = cpool.tile([1, HID_DIM], mdt)
            w2l_sb = cpool.tile([HID_DIM, OUT_DIM], mdt)
            w2r_sb = cpool.tile([HID_DIM, OUT_DIM], f32)
            b2_sb = cpool.tile([1, OUT_DIM], mdt)
            ones_sb = cpool.tile([1, W], mdt)
            iota_sb = cpool.tile([128, W], tdt)

            nc.sync.dma_start(out=idx_sb[:], in_=idx16[:])
            nc.sync.dma_start(out=dst_sb[:], in_=dstloc[:])
            nc.sync.dma_start(out=cnt_sb[:], in_=cntinv[:])
            nc.sync.dma_start(out=xT_sb[:], in_=xT[:])
            nc.sync.dma_start(out=w1l_sb[:], in_=w1l[:])
            nc.sync.dma_start(out=w1r_sb[:], in_=w1r[:])
            nc.sync.dma_start(out=b1_sb[:], in_=b1row[:])
            nc.sync.dma_start(out=w2l_sb[:], in_=w2l[:])
            nc.sync.dma_start(out=w2r_sb[:], in_=w2r[:])
            nc.sync.dma_start(out=b2_sb[:], in_=b2row[:])
            nc.sync.dma_start(out=iota_sb[:], in_=iota_in[:])
            nc.sync.dma_start(out=ones_sb[:], in_=onesr[:])

            def emit_cc():
                nc.gpsimd.collective_compute(
                    "AllGather",
                    mybir.AluOpType.bypass,
                    replica_groups=[list(range(n_cores))],
                    ins=[h_local[:].opt()],
                    outs=[h_full[:].opt()],
                )

            # A collective inside a hardware For loop desyncs the mesh at
            # runtime, so loop_k>1 (timing-only) builds run the AllGather
            # once before the loop: identical DMA/compute stream per
            # iteration, only the h_full values differ.
            if loop_k > 1 and "c" in _STAGE_MASK:
                emit_cc()

            loop_ctx = (tc.For_i(0, loop_k, 1) if loop_k > 1
                        else contextlib.nullcontext())
            stack = contextlib.ExitStack()
            stack.enter_context(loop_ctx)

            gq = [0]  # round-robin SWDGE queue cursor

            def run_layer(layer, table, out_blk):
                gtiles = [dict() for _ in range(n_chunks)]
                next_g = [0] * n_chunks

                def ensure_gather(c, gi):
                    while next_g[c] <= gi:
                        g = next_g[c]
                        s0, n = gathers[c][g]
                        gb = gpool.tile([128, GATHER_MAX // 128, D], tdt,
                                        tag=f"g{c}",
                                        name=f"gb{layer}_{c}_{g}")
                        nc.gpsimd.dma_gather(
                            out_ap=gb[:, : -(-n // 128), :],
                            in_ap=table[c * chunk_sz:(c + 1) * chunk_sz, :],
                            idxs_ap=idx_sb[:, s0 // 16:(s0 + n) // 16],
                            num_idxs=n,
                            num_idxs_reg=n,
                            elem_size=D,
                            queue_num=gq[0] % N_QUEUES,
                        )
                        gq[0] += 1
                        gtiles[c][g] = gb
                        next_g[c] = g + 1

                for b in range(bpc):
                    agg = psA.tile([D, W], f32, space="PSUM",
                                   tag="agg", name=f"agg{layer}_{b}")
                    n_mm = int(T[b].sum())
                    mm = 0
                    for c in range(n_chunks):
                        for t in range(int(T[b, c])):
                            slot0 = int(cell_base[b, c]) + t * 128
                            g = (slot0 - int(seg_start[c])) // GATHER_MAX
                            tin = ((slot0 - int(seg_start[c]))
                                   % GATHER_MAX) // 128
                            ensure_gather(c, g)
                            gb = gtiles[c][g]
                            gt_col = slot0 // 128
                            s_tile = spool.tile([128, W], tdt, tag="s",
                                                name=f"s{layer}_{b}_{c}_{t}")
                            nc.vector.tensor_scalar(
                                out=s_tile[:],
                                in0=iota_sb[:],
                                scalar1=dst_sb[:, gt_col:gt_col + 1],
                                scalar2=cnt_sb[:, gt_col:gt_col + 1],
                                op0=mybir.AluOpType.is_equal,
                                op1=mybir.AluOpType.mult,
                            )
                            nc.tensor.matmul(
                                out=agg[:],
                                lhsT=gb[:, tin, :],
                                rhs=s_tile[:],
                                start=(mm == 0),
                                stop=(mm == n_mm - 1),
                            )
                            mm += 1

                    aggc = None
                    if n_mm > 0:
                        aggc = fpool.tile([D, W], mdt, tag="aggc",
                                          name=f"aggc{layer}_{b}")
                        nc.scalar.copy(out=aggc[:], in_=agg[:])
                    out_blk(b, aggc)

            # ---------------- Layer 1 ----------------
            def l1_out(b, aggc):
                # transposed finalize -> relu -> hT_sb columns
                outpT = psB.tile([HID_DIM, W], f32, space="PSUM",
                                 tag="outpT", name=f"outpT_{b}")
                if aggc is not None:
                    nc.tensor.matmul(out=outpT[:], lhsT=w1l_sb[:],
                                     rhs=aggc[:], start=True, stop=False)
                nc.tensor.matmul(out=outpT[:], lhsT=w1r_sb[:],
                                 rhs=xT_sb[:, b * W:(b + 1) * W],
                                 start=(aggc is None), stop=False)
                nc.tensor.matmul(out=outpT[:], lhsT=b1_sb[:],
                                 rhs=ones_sb[:], start=False, stop=True)
                nc.vector.tensor_scalar(
                    out=hT_sb[:, b * W:(b + 1) * W], in0=outpT[:],
                    scalar1=0.0, scalar2=None, op0=mybir.AluOpType.max)
                # row-major finalize -> relu -> h_local rows (gather table)
                for hh in range(2):
                    sl = slice(b * W + hh * 128, b * W + (hh + 1) * 128)
                    outr = psC.tile([128, HID_DIM], f32, space="PSUM",
                                    tag="outr", name=f"outr_{b}_{hh}")
                    if aggc is not None:
                        nc.tensor.matmul(
                            out=outr[:],
                            lhsT=aggc[:, hh * 128:(hh + 1) * 128],
                            rhs=w1l_sb[:], start=True, stop=False)
                    nc.tensor.matmul(out=outr[:], lhsT=xT_sb[:, sl],
                                     rhs=w1r_sb[:],
                                     start=(aggc is None), stop=False)
                    nc.tensor.matmul(out=outr[:], lhsT=ones_sb[:, :128],
                                     rhs=b1_sb[:], start=False, stop=True)
                    finr = fpool.tile([128, HID_DIM], tdt, tag="finr",
                                      name=f"finr_{b}_{hh}")
                    nc.vector.tensor_scalar(
                        out=finr[:], in0=outr[:], scalar1=0.0,
                        scalar2=None, op0=mybir.AluOpType.max)
                    nc.sync.dma_start(out=h_local[sl, :], in_=finr[:])

            if "1" in _STAGE_MASK:
                run_layer(1, xq, l1_out)

            # ---------------- exchange ----------------
            if "c" in _STAGE_MASK and loop_k == 1:
                emit_cc()

            # ---------------- Layer 2 ----------------
            def l2_out(b, aggc):
                outp2 = psB.tile([OUT_DIM, W], f32, space="PSUM",
                                 tag="outp2", name=f"outp2_{b}")
                if aggc is not None:
                    nc.tensor.matmul(out=outp2[:], lhsT=w2l_sb[:],
                                     rhs=aggc[:], start=True, stop=False)
                nc.tensor.matmul(out=outp2[:], lhsT=w2r_sb[:],
                                 rhs=hT_sb[:, b * W:(b + 1) * W],
                                 start=(aggc is None), stop=False)
                nc.tensor.matmul(out=outp2[:], lhsT=b2_sb[:],
                                 rhs=ones_sb[:], start=False, stop=True)
                fin2 = fpool.tile([OUT_DIM, W], f32, tag="fin2",
                                  name=f"fin2_{b}")
                nc.vector.tensor_copy(out=fin2[:], in_=outp2[:])
                nc.sync.dma_start(out=out[:, b * W:(b + 1) * W],
                                  in_=fin2[:])

            if "2" in _STAGE_MASK:
                run_layer(2, h_full, l2_out)
            stack.close()

    nc.compile()
    names = dict(xq=xq.name, idx16=idx16.name, dstloc=dstloc.name,
                 cntinv=cntinv.name, xT=xT.name,
                 w1l=w1l.name, w1r=w1r.name, b1row=b1row.name,
                 w2l=w2l.name, w2r=w2r.name, b2row=b2row.name,
                 iota=iota_in.name, onesr=onesr.name, out=out.name)
    return nc, names


def _build_cc_program(plan, n_cc):
    """Timing-only: n_cc back-to-back AllGathers of the hidden exchange."""
    spc = plan["slots_per_core"]
    total_rows = plan["total_rows"]
    n_cores = plan["n_cores"]
    D = 128
    f32 = mybir.dt.float32
    tdt = mybir.dt.bfloat16 if MSG_BF16 else f32

    nc = bacc.Bacc("TRN2", target_bir_lowering=False, debug=False)
    with tile.TileContext(nc) as tc:
        with tc.tile_pool(name="dram", bufs=1, space="DRAM") as dram:
            h_seed = dram.tile([spc, D], tdt,
                               kind="ExternalInput", name="h_seed")
            out = dram.tile([1, D], tdt, kind="ExternalOutput", name="out")
            h_local = dram.tile([spc, D], tdt, name="h_local")
            h_full = dram.tile([total_rows, D], tdt, name="h_full")
        with tc.tile_pool(name="sb", bufs=1) as sb:
            row = sb.tile([1, D], tdt)
            nc.sync.dma_start(out=row[:], in_=h_seed[0:1, :])
            nc.sync.dma_start(out=h_local[0:1, :], in_=row[:])
            for _ in range(n_cc):
                nc.gpsimd.collective_compute(
                    "AllGather",
                    mybir.AluOpType.bypass,
                    replica_groups=[list(range(n_cores))],
                    ins=[h_local[:].opt()],
                    outs=[h_full[:].opt()],
                )
            nc.sync.dma_start(out=row[:], in_=h_full[total_rows - 1:, :])
            nc.sync.dma_start(out=out[:], in_=row[:])
    nc.compile()
    return nc, dict(h_seed=h_seed.name, out=out.name)


def _in_maps(names, plan, xq_np, xT_np, w1lT, w1rT, b1, w2lT, w2rT, b2):
    iota = np.broadcast_to(np.arange(W, dtype=np.float32), (128, W)).copy()
    if MSG_BF16:
        iota = iota.astype(mybir.dt.np(mybir.dt.bfloat16))
    in_maps = []
    for c in range(plan["n_cores"]):
        in_maps.append({
            names["xq"]: xq_np,
            names["idx16"]: plan["idx16"][c],
            names["dstloc"]: plan["dstloc"][c],
            names["cntinv"]: plan["cntinv"][c],
            names["xT"]: xT_np[c],
            names["w1l"]: w1lT,
            names["w1r"]: w1rT,
            names["b1row"]: np.ascontiguousarray(b1.reshape(1, HID_DIM)),
            names["w2l"]: w2lT,
            names["w2r"]: w2rT,
            names["b2row"]: np.ascontiguousarray(b2.reshape(1, OUT_DIM)),
            names["iota"]: iota,
            names["onesr"]: np.ones((1, W), np.float32),
        })
    return in_maps


def _get_plan_and_prog(edge_index):
    key = hash(edge_index.tobytes())
    if key not in _plan_cache:
        _plan_cache[key] = _make_plan(edge_index)
    plan = _plan_cache[key]
    if key not in _prog_cache:
        _prog_cache[key] = _build_program(plan)
    return plan, _prog_cache[key]


def _host_inputs(plan, x, W1l, b1, W1r, W2l, b2, W2r):
    spc = plan["slots_per_core"]
    n_cores = plan["n_cores"]
    xq = np.zeros((plan["total_rows"], IN_DIM), np.float32)
    xq[plan["slot_of_node"]] = x
    xT_np = [np.ascontiguousarray(xq[c * spc:(c + 1) * spc].T)
             for c in range(n_cores)]
    if MSG_BF16:
        xq = xq.astype(mybir.dt.np(mybir.dt.bfloat16))
    return dict(
        xq_np=xq, xT_np=xT_np,
        w1lT=np.ascontiguousarray(W1l.T), w1rT=np.ascontiguousarray(W1r.T),
        b1=b1,
        w2lT=np.ascontiguousarray(W2l.T), w2rT=np.ascontiguousarray(W2r.T),
        b2=b2,
    )


def kernel(x, edge_index, W1l, b1, W1r, W2l, b2, W2r):
    x = np.asarray(x, np.float32)
    edge_index = np.asarray(edge_index)
    plan, (nc, names) = _get_plan_and_prog(edge_index)

    hi = _host_inputs(plan, x, np.asarray(W1l, np.float32),
                      np.asarray(b1, np.float32),
                      np.asarray(W1r, np.float32),
                      np.asarray(W2l, np.float32),
                      np.asarray(b2, np.float32),
                      np.asarray(W2r, np.float32))
    in_maps = _in_maps(names, plan, **hi)
    res = bass_utils.run_bass_kernel_spmd(
        nc, in_maps, core_ids=list(range(plan["n_cores"])))
    out_parts = [res.results[c][names["out"]]
                 for c in range(plan["n_cores"])]

    oq = np.concatenate(out_parts, axis=1)        # [out_d, total_rows]
    return np.ascontiguousarray(oq.T[plan["slot_of_node"]]).astype(
        np.float32)


# revision 22
# speedup vs baseline: 1.4124x; 1.4124x over previous
"""2-layer GraphSAGE (mean aggr) on 8 Trainium2 NeuronCores — single launch.

Strategy: partition destination nodes across cores (graph parallel), both
layers fused into ONE program with an on-device AllGather exchanging the
hidden layer between them.

All tables live in "slot space": host permutes nodes into degree-balanced
slots (degree-sorted snake round-robin over blocks), core c owning slots
[c*spc, (c+1)*spc). Because both the L1 input table (xq) and the L2 input
table (h_full) are slot-ordered with the same chunking, one set of gather
indices / routing tensors serves both layers.

Per layer, per block of W=256 destination slots: gather tiles of 128
source rows M [128e, D] with dma_gather (int16 idx, 4 chunks of 25088
rows, round-robin over 4 SWDGE queues), build routing tile S [128e, W]
(one-hot by local destination x 1/deg) on DVE, accumulate agg[D, W] +=
M.T @ S on TensorE (fp32r, 1 cycle/row) in PSUM. Finalize twice: a
transposed [HID, W] tile (relu -> SBUF hT, the L2 self term) and a
row-major [W, HID] pair (relu -> DRAM h_local rows, the L2 gather table).
AllGather h_local [spc, D] -> h_full [8*spc, D], then L2 runs the same
block loop over h_full and writes out [64, spc] transposed; host
assembles/unpermutes.
"""

import contextlib
import sys

sys.path.insert(0, "/opt/trn_rl_repo")

import numpy as np

import concourse.mybir as mybir
import concourse.tile as tile
from concourse import bacc, bass_utils

N_NODES = 100000
N_EDGES = 1600000
IN_DIM = 128
HID_DIM = 128
OUT_DIM = 64
N_CORES = 8
N_CHUNKS = 4
W = 256               # destination slots per block (fp32r: >=256 moving rows)
GATHER_MAX = 1024     # HW limit: dma_gather wedges above this
N_QUEUES = 4          # SWDGE queues (ucode max 4)
MSG_BF16 = True       # message tables (xq, h) + routing tiles in bf16

_plan_cache: dict = {}
_prog_cache: dict = {}
_STAGE_MASK = "1c2"   # experiment hook: subset of stages to emit


def _make_plan(edge_index, n_nodes=N_NODES, n_cores=N_CORES,
               n_chunks=N_CHUNKS):
    src = np.asarray(edge_index[0], dtype=np.int64)
    dst = np.asarray(edge_index[1], dtype=np.int64)
    n_edges = src.shape[0]

    deg = np.bincount(dst, minlength=n_nodes).astype(np.int64)
    cnt_inv = (1.0 / np.maximum(deg, 1)).astype(np.float32)

    # Balanced blocks: degree-sorted snake round-robin over all blocks.
    n_blocks_total = -(-n_nodes // W)
    while n_blocks_total % n_cores:
        n_blocks_total += 1
    bpc = n_blocks_total // n_cores
    slots_per_core = bpc * W
    total_rows = n_blocks_total * W          # slot-space table height
    chunk_sz = total_rows // n_chunks
    assert chunk_sz - 1 <= np.iinfo(np.int16).max

    order = np.argsort(-deg, kind="stable")
    i = np.arange(n_nodes)
    r = i // n_blocks_total
    b = i % n_blocks_total
    b = np.where(r % 2 == 0, b, n_blocks_total - 1 - b)
    slot = b * W + r
    slot_of_node = np.empty(n_nodes, np.int64)
    slot_of_node[order] = slot

    sslot = slot_of_node[src]                # gather index, slot space
    dslot = slot_of_node[dst]
    core_e = dslot // slots_per_core
    blk_e = (dslot % slots_per_core) // W
    dloc_e = dslot % W
    chunk_e = sslot // chunk_sz

    cell = (core_e * bpc + blk_e) * n_chunks + chunk_e
    n_cells = n_cores * bpc * n_chunks
    counts = np.bincount(cell, minlength=n_cells).reshape(
        n_cores, bpc, n_chunks)
    # Exact packing: cell size = max edge count over cores (no roundup).
    # Tiles on the global 128 grid may straddle two cells; each straddled
    # tile gets one matmul entry per block, with the other block's rows
    # zeroed via dloc=-1 in that entry's dst/cnt column.
    n_cell = counts.max(axis=0).astype(np.int64)     # [bpc, n_chunks]
    seg_len = n_cell.sum(axis=0)
    seg_len = -(-seg_len // 128) * 128               # chunk streams 128-pad
    seg_start = np.concatenate([[0], np.cumsum(seg_len)[:-1]])
    cell_base = np.empty((bpc, n_chunks), np.int64)
    for c in range(n_chunks):
        cell_base[:, c] = seg_start[c] + np.concatenate(
            [[0], np.cumsum(n_cell[:, c])[:-1]])
    total_slots = int(seg_len.sum())

    gathers = []
    for c in range(n_chunks):
        lst = []
        off = 0
        while off < seg_len[c]:
            n = int(min(GATHER_MAX, seg_len[c] - off))
            lst.append((int(seg_start[c] + off), n))
            off += n
        gathers.append(lst)

    # slot position of every edge
    eorder = np.argsort(cell, kind="stable")
    sorted_cell = cell[eorder]
    group_start = np.zeros(n_edges, np.int64)
    new_grp = np.empty(n_edges, bool)
    new_grp[0] = True
    new_grp[1:] = sorted_cell[1:] != sorted_cell[:-1]
    grp_first = np.where(new_grp)[0]
    group_start[grp_first] = grp_first
    group_start = np.maximum.accumulate(group_start)
    rank = np.arange(n_edges) - group_start

    b_of = (sorted_cell // n_chunks) % bpc
    c_of = sorted_cell % n_chunks
    core_of = sorted_cell // (bpc * n_chunks)
    pos = cell_base[b_of, c_of] + rank

    idx_vals = np.zeros((n_cores, total_slots), np.int16)
    dloc_vals = np.full((n_cores, total_slots), -1.0, np.float32)
    cinv_vals = np.zeros((n_cores, total_slots), np.float32)

    es, ed = sslot[eorder], dst[eorder]
    idx_vals[core_of, pos] = (es - c_of * chunk_sz).astype(np.int16)
    dloc_vals[core_of, pos] = dloc_e[eorder].astype(np.float32)
    cinv_vals[core_of, pos] = cnt_inv[ed]

    # matmul entries: per block, (gt_col, scol) pairs. gt_col indexes the
    # gather stream's global 128-tile grid; scol indexes the entry-major
    # dst/cnt arrays (one column per entry, -1 outside the cell range).
    entries = [[] for _ in range(bpc)]
    dcols = []
    for c in range(n_chunks):
        for b in range(bpc):
            lo = int(cell_base[b, c])
            hi = lo + int(n_cell[b, c])
            for t0 in range(lo // 128, -(-hi // 128)):
                r0 = max(lo, 128 * t0)
                r1 = min(hi, 128 * (t0 + 1))
                entries[b].append((t0, len(dcols)))
                dcols.append((t0, r0, r1))

    n_entries = len(dcols)
    dst_e = np.full((n_cores, 128, n_entries), -1.0, np.float32)
    cnt_e = np.zeros((n_cores, 128, n_entries), np.float32)
    for scol, (t0, r0, r1) in enumerate(dcols):
        dst_e[:, r0 - 128 * t0:r1 - 128 * t0, scol] = dloc_vals[:, r0:r1]
        cnt_e[:, r0 - 128 * t0:r1 - 128 * t0, scol] = cinv_vals[:, r0:r1]
    dstloc = np.ascontiguousarray(dst_e)
    cntinv = np.ascontiguousarray(cnt_e)

    idx16 = np.ascontiguousarray(
        np.tile(idx_vals.reshape(n_cores, -1, 16).transpose(0, 2, 1),
                (1, 8, 1)))

    return dict(
        slot_of_node=slot_of_node, bpc=bpc, slots_per_core=slots_per_core,
        total_rows=total_rows, chunk_sz=chunk_sz,
        gathers=gathers, total_slots=total_slots, entries=entries,
        n_entries=n_entries, seg_start=seg_start,
        idx16=idx16, dstloc=dstloc, cntinv=cntinv,
        n_chunks=n_chunks, n_nodes=n_nodes, n_cores=n_cores,
    )


def _build_program(plan, loop_k=1):
    """The fused two-layer SPMD program (shared by all cores).

    loop_k > 1 wraps the whole body (both layers + AllGather) in a
    hardware For loop repeating it loop_k times (timing only).
    """
    bpc = plan["bpc"]
    entries = plan["entries"]
    n_entries = plan["n_entries"]
    n_chunks = plan["n_chunks"]
    chunk_sz = plan["chunk_sz"]
    total_slots = plan["total_slots"]
    total_rows = plan["total_rows"]
    spc = plan["slots_per_core"]
    gathers = plan["gathers"]
    seg_start = plan["seg_start"]
    n_cores = plan["n_cores"]

    def chunk_of_tile(gt):
        c = 0
        while c + 1 < n_chunks and gt * 128 >= seg_start[c + 1]:
            c += 1
        return c
    D = 128
    f32 = mybir.dt.float32
    mdt = mybir.dt.float32r
    tdt = mybir.dt.bfloat16 if MSG_BF16 else mdt   # gather-table dtype

    nc = bacc.Bacc("TRN2", target_bir_lowering=False, debug=False,
                   num_swdge_queues=N_QUEUES)
    with tile.TileContext(nc) as tc:
        with tc.tile_pool(name="dram", bufs=1, space="DRAM") as dram:
            xq = dram.tile([total_rows, D], tdt,
                           kind="ExternalInput", name="xq")
            idx16 = dram.tile([128, total_slots // 16], mybir.dt.int16,
                              kind="ExternalInput", name="idx16")
            dstloc = dram.tile([128, n_entries], f32,
                               kind="ExternalInput", name="dstloc")
            cntinv = dram.tile([128, n_entries], f32,
                               kind="ExternalInput", name="cntinv")
            xT = dram.tile([D, spc], f32, kind="ExternalInput", name="xT")
            w1l = dram.tile([D, HID_DIM], mdt,
                            kind="ExternalInput", name="w1l")
            w1r = dram.tile([D, HID_DIM], f32,
                            kind="ExternalInput", name="w1r")
            b1row = dram.tile([1, HID_DIM], mdt,
                              kind="ExternalInput", name="b1row")
            w2l = dram.tile([HID_DIM, OUT_DIM], mdt,
                            kind="ExternalInput", name="w2l")
            w2r = dram.tile([HID_DIM, OUT_DIM], f32,
                            kind="ExternalInput", name="w2r")
            b2row = dram.tile([1, OUT_DIM], mdt,
                              kind="ExternalInput", name="b2row")
            iota_in = dram.tile([128, W], tdt,
                                kind="ExternalInput", name="iota")
            onesr = dram.tile([1, W], mdt,
                              kind="ExternalInput", name="onesr")
            out = dram.tile([OUT_DIM, spc], f32,
                            kind="ExternalOutput", name="out")
            h_local = dram.tile([spc, D], tdt, name="h_local")
            h_full = dram.tile([total_rows, D], tdt, name="h_full",
                               addr_space="Shared")

        with tc.tile_pool(name="const", bufs=1) as cpool, \
             tc.tile_pool(name="gbuf", bufs=2) as gpool, \
             tc.tile_pool(name="spool", bufs=4) as spool, \
             tc.tile_pool(name="fpool", bufs=4) as fpool, \
             tc.tile_pool(name="psA", bufs=2, space="PSUM") as psA, \
             tc.tile_pool(name="psB", bufs=2, space="PSUM") as psB, \
             tc.tile_pool(name="psC", bufs=2, space="PSUM") as psC:

            idx_sb = cpool.tile([128, total_slots // 16], mybir.dt.int16)
            dst_sb = cpool.tile([128, n_entries], f32)
            cnt_sb = cpool.tile([128, n_entries], f32)
            xT_sb = cpool.tile([D, spc], f32)
            hT_sb = cpool.tile([D, spc], f32)
            w1l_sb = cpool.tile([D, HID_DIM], mdt)
            w1r_sb = cpool.tile([D, HID_DIM], f32)
            b1_sb = cpool.tile([1, HID_DIM], mdt)
            w2l_sb = cpool.tile([HID_DIM, OUT_DIM], mdt)
            w2r_sb = cpool.tile([HID_DIM, OUT_DIM], f32)
            b2_sb = cpool.tile([1, OUT_DIM], mdt)
            ones_sb = cpool.tile([1, W], mdt)
            iota_sb = cpool.tile([128, W], tdt)

            nc.sync.dma_start(out=idx_sb[:], in_=idx16[:])
            nc.sync.dma_start(out=dst_sb[:], in_=dstloc[:])
            nc.sync.dma_start(out=cnt_sb[:], in_=cntinv[:])
            nc.sync.dma_start(out=xT_sb[:], in_=xT[:])
            nc.sync.dma_start(out=w1l_sb[:], in_=w1l[:])
            nc.sync.dma_start(out=w1r_sb[:], in_=w1r[:])
            nc.sync.dma_start(out=b1_sb[:], in_=b1row[:])
            nc.sync.dma_start(out=w2l_sb[:], in_=w2l[:])
            nc.sync.dma_start(out=w2r_sb[:], in_=w2r[:])
            nc.sync.dma_start(out=b2_sb[:], in_=b2row[:])
            nc.sync.dma_start(out=iota_sb[:], in_=iota_in[:])
            nc.sync.dma_start(out=ones_sb[:], in_=onesr[:])

            def emit_cc():
                nc.gpsimd.collective_compute(
                    "AllGather",
                    mybir.AluOpType.bypass,
                    replica_groups=[list(range(n_cores))],
                    ins=[h_local[:].opt()],
                    outs=[h_full[:].opt()],
                )

            # A collective inside a hardware For loop desyncs the mesh at
            # runtime, so loop_k>1 (timing-only) builds run the AllGather
            # once before the loop: identical DMA/compute stream per
            # iteration, only the h_full values differ.
            if loop_k > 1 and "c" in _STAGE_MASK:
                emit_cc()

            loop_ctx = (tc.For_i(0, loop_k, 1) if loop_k > 1
                        else contextlib.nullcontext())
            stack = contextlib.ExitStack()
            stack.enter_context(loop_ctx)

            gq = [0]  # round-robin SWDGE queue cursor

            def run_layer(layer, table, out_blk):
                gtiles = [dict() for _ in range(n_chunks)]
                next_g = [0] * n_chunks

                def ensure_gather(c, gi):
                    while next_g[c] <= gi:
                        g = next_g[c]
                        s0, n = gathers[c][g]
                        gb = gpool.tile([128, GATHER_MAX // 128, D], tdt,
                                        tag=f"g{c}",
                                        name=f"gb{layer}_{c}_{g}")
                        nc.gpsimd.dma_gather(
                            out_ap=gb[:, : -(-n // 128), :],
                            in_ap=table[c * chunk_sz:(c + 1) * chunk_sz, :],
                            idxs_ap=idx_sb[:, s0 // 16:(s0 + n) // 16],
                            num_idxs=n,
                            num_idxs_reg=n,
                            elem_size=D,
                            queue_num=gq[0] % N_QUEUES,
                        )
                        gq[0] += 1
                        gtiles[c][g] = gb
                        next_g[c] = g + 1

                for b in range(bpc):
                    agg = psA.tile([D, W], f32, space="PSUM",
                                   tag="agg", name=f"agg{layer}_{b}")
                    ents = entries[b]
                    n_mm = len(ents)
                    for mm, (gt, scol) in enumerate(ents):
                        c = chunk_of_tile(gt)
                        off = gt * 128 - int(seg_start[c])
                        g = off // GATHER_MAX
                        tin = (off % GATHER_MAX) // 128
                        ensure_gather(c, g)
                        gb = gtiles[c][g]
                        s_tile = spool.tile([128, W], tdt, tag="s",
                                            name=f"s{layer}_{b}_{mm}")
                        nc.vector.tensor_scalar(
                            out=s_tile[:],
                            in0=iota_sb[:],
                            scalar1=dst_sb[:, scol:scol + 1],
                            scalar2=cnt_sb[:, scol:scol + 1],
                            op0=mybir.AluOpType.is_equal,
                            op1=mybir.AluOpType.mult,
                        )
                        nc.tensor.matmul(
                            out=agg[:],
                            lhsT=gb[:, tin, :],
                            rhs=s_tile[:],
                            start=(mm == 0),
                            stop=(mm == n_mm - 1),
                        )

                    aggc = None
                    if n_mm > 0:
                        aggc = fpool.tile([D, W], mdt, tag="aggc",
                                          name=f"aggc{layer}_{b}")
                        nc.scalar.copy(out=aggc[:], in_=agg[:])
                    out_blk(b, aggc)

            # ---------------- Layer 1 ----------------
            def l1_out(b, aggc):
                # transposed finalize -> relu -> hT_sb columns
                outpT = psB.tile([HID_DIM, W], f32, space="PSUM",
                                 tag="outpT", name=f"outpT_{b}")
                if aggc is not None:
                    nc.tensor.matmul(out=outpT[:], lhsT=w1l_sb[:],
                                     rhs=aggc[:], start=True, stop=False)
                nc.tensor.matmul(out=outpT[:], lhsT=w1r_sb[:],
                                 rhs=xT_sb[:, b * W:(b + 1) * W],
                                 start=(aggc is None), stop=False)
                nc.tensor.matmul(out=outpT[:], lhsT=b1_sb[:],
                                 rhs=ones_sb[:], start=False, stop=True)
                nc.vector.tensor_scalar(
                    out=hT_sb[:, b * W:(b + 1) * W], in0=outpT[:],
                    scalar1=0.0, scalar2=None, op0=mybir.AluOpType.max)
                # row-major finalize -> relu -> h_local rows (gather table)
                for hh in range(2):
                    sl = slice(b * W + hh * 128, b * W + (hh + 1) * 128)
                    outr = psC.tile([128, HID_DIM], f32, space="PSUM",
                                    tag="outr", name=f"outr_{b}_{hh}")
                    if aggc is not None:
                        nc.tensor.matmul(
                            out=outr[:],
                            lhsT=aggc[:, hh * 128:(hh + 1) * 128],
                            rhs=w1l_sb[:], start=True, stop=False)
                    nc.tensor.matmul(out=outr[:], lhsT=xT_sb[:, sl],
                                     rhs=w1r_sb[:],
                                     start=(aggc is None), stop=False)
                    nc.tensor.matmul(out=outr[:], lhsT=ones_sb[:, :128],
                                     rhs=b1_sb[:], start=False, stop=True)
                    finr = fpool.tile([128, HID_DIM], tdt, tag="finr",
                                      name=f"finr_{b}_{hh}")
                    nc.vector.tensor_scalar(
                        out=finr[:], in0=outr[:], scalar1=0.0,
                        scalar2=None, op0=mybir.AluOpType.max)
                    nc.sync.dma_start(out=h_local[sl, :], in_=finr[:])

            if "1" in _STAGE_MASK:
                run_layer(1, xq, l1_out)

            # ---------------- exchange ----------------
            if "c" in _STAGE_MASK and loop_k == 1:
                emit_cc()

            # ---------------- Layer 2 ----------------
            def l2_out(b, aggc):
                outp2 = psB.tile([OUT_DIM, W], f32, space="PSUM",
                                 tag="outp2", name=f"outp2_{b}")
                if aggc is not None:
                    nc.tensor.matmul(out=outp2[:], lhsT=w2l_sb[:],
                                     rhs=aggc[:], start=True, stop=False)
                nc.tensor.matmul(out=outp2[:], lhsT=w2r_sb[:],
                                 rhs=hT_sb[:, b * W:(b + 1) * W],
                                 start=(aggc is None), stop=False)
                nc.tensor.matmul(out=outp2[:], lhsT=b2_sb[:],
                                 rhs=ones_sb[:], start=False, stop=True)
                fin2 = fpool.tile([OUT_DIM, W], f32, tag="fin2",
                                  name=f"fin2_{b}")
                nc.vector.tensor_copy(out=fin2[:], in_=outp2[:])
                nc.sync.dma_start(out=out[:, b * W:(b + 1) * W],
                                  in_=fin2[:])

            if "2" in _STAGE_MASK:
                run_layer(2, h_full, l2_out)
            stack.close()

    nc.compile()
    names = dict(xq=xq.name, idx16=idx16.name, dstloc=dstloc.name,
                 cntinv=cntinv.name, xT=xT.name,
                 w1l=w1l.name, w1r=w1r.name, b1row=b1row.name,
                 w2l=w2l.name, w2r=w2r.name, b2row=b2row.name,
                 iota=iota_in.name, onesr=onesr.name, out=out.name)
    return nc, names


def _build_cc_program(plan, n_cc):
    """Timing-only: n_cc back-to-back AllGathers of the hidden exchange."""
    spc = plan["slots_per_core"]
    total_rows = plan["total_rows"]
    n_cores = plan["n_cores"]
    D = 128
    f32 = mybir.dt.float32
    tdt = mybir.dt.bfloat16 if MSG_BF16 else f32

    nc = bacc.Bacc("TRN2", target_bir_lowering=False, debug=False)
    with tile.TileContext(nc) as tc:
        with tc.tile_pool(name="dram", bufs=1, space="DRAM") as dram:
            h_seed = dram.tile([spc, D], tdt,
                               kind="ExternalInput", name="h_seed")
            out = dram.tile([1, D], tdt, kind="ExternalOutput", name="out")
            h_local = dram.tile([spc, D], tdt, name="h_local")
            h_full = dram.tile([total_rows, D], tdt, name="h_full",
                               addr_space="Shared")
        with tc.tile_pool(name="sb", bufs=1) as sb:
            row = sb.tile([1, D], tdt)
            nc.sync.dma_start(out=row[:], in_=h_seed[0:1, :])
            nc.sync.dma_start(out=h_local[0:1, :], in_=row[:])
            for _ in range(n_cc):
                nc.gpsimd.collective_compute(
                    "AllGather",
                    mybir.AluOpType.bypass,
                    replica_groups=[list(range(n_cores))],
                    ins=[h_local[:].opt()],
                    outs=[h_full[:].opt()],
                )
            nc.sync.dma_start(out=row[:], in_=h_full[total_rows - 1:, :])
            nc.sync.dma_start(out=out[:], in_=row[:])
    nc.compile()
    return nc, dict(h_seed=h_seed.name, out=out.name)


def _in_maps(names, plan, xq_np, xT_np, w1lT, w1rT, b1, w2lT, w2rT, b2):
    iota = np.broadcast_to(np.arange(W, dtype=np.float32), (128, W)).copy()
    if MSG_BF16:
        iota = iota.astype(mybir.dt.np(mybir.dt.bfloat16))
    in_maps = []
    for c in range(plan["n_cores"]):
        in_maps.append({
            names["xq"]: xq_np,
            names["idx16"]: plan["idx16"][c],
            names["dstloc"]: plan["dstloc"][c],
            names["cntinv"]: plan["cntinv"][c],
            names["xT"]: xT_np[c],
            names["w1l"]: w1lT,
            names["w1r"]: w1rT,
            names["b1row"]: np.ascontiguousarray(b1.reshape(1, HID_DIM)),
            names["w2l"]: w2lT,
            names["w2r"]: w2rT,
            names["b2row"]: np.ascontiguousarray(b2.reshape(1, OUT_DIM)),
            names["iota"]: iota,
            names["onesr"]: np.ones((1, W), np.float32),
        })
    return in_maps


def _get_plan_and_prog(edge_index):
    key = hash(edge_index.tobytes())
    if key not in _plan_cache:
        _plan_cache[key] = _make_plan(edge_index)
    plan = _plan_cache[key]
    if key not in _prog_cache:
        _prog_cache[key] = _build_program(plan)
    return plan, _prog_cache[key]


def _host_inputs(plan, x, W1l, b1, W1r, W2l, b2, W2r):
    spc = plan["slots_per_core"]
    n_cores = plan["n_cores"]
    xq = np.zeros((plan["total_rows"], IN_DIM), np.float32)
    xq[plan["slot_of_node"]] = x
    xT_np = [np.ascontiguousarray(xq[c * spc:(c + 1) * spc].T)
             for c in range(n_cores)]
    if MSG_BF16:
        xq = xq.astype(mybir.dt.np(mybir.dt.bfloat16))
    return dict(
        xq_np=xq, xT_np=xT_np,
        w1lT=np.ascontiguousarray(W1l.T), w1rT=np.ascontiguousarray(W1r.T),
        b1=b1,
        w2lT=np.ascontiguousarray(W2l.T), w2rT=np.ascontiguousarray(W2r.T),
        b2=b2,
    )


def kernel(x, edge_index, W1l, b1, W1r, W2l, b2, W2r):
    x = np.asarray(x, np.float32)
    edge_index = np.asarray(edge_index)
    plan, (nc, names) = _get_plan_and_prog(edge_index)

    hi = _host_inputs(plan, x, np.asarray(W1l, np.float32),
                      np.asarray(b1, np.float32),
                      np.asarray(W1r, np.float32),
                      np.asarray(W2l, np.float32),
                      np.asarray(b2, np.float32),
                      np.asarray(W2r, np.float32))
    in_maps = _in_maps(names, plan, **hi)
    res = bass_utils.run_bass_kernel_spmd(
        nc, in_maps, core_ids=list(range(plan["n_cores"])))
    out_parts = [res.results[c][names["out"]]
                 for c in range(plan["n_cores"])]

    oq = np.concatenate(out_parts, axis=1)        # [out_d, total_rows]
    return np.ascontiguousarray(oq.T[plan["slot_of_node"]]).astype(
        np.float32)
